# revision 17
# baseline (speedup 1.0000x reference)
"""Trainium2 Bass kernel for nn_CYActorNetwork (GCN x3 -> MHA -> global mean -> MLP head).

Strategy (8 NeuronCores, SPMD):
  - Graph preprocessing on host: edge_index -> dense normalized adjacency
    A_norm = D^-1/2 (Adj + I) D^-1/2  (exactly matches PyG GCNConv with
    self-loops + symmetric normalization, including duplicate-edge
    multiplicity). Each core gets the transposed column-slice A_norm.T[:, rows_r]
    so the per-layer aggregation  out_r = A_norm[rows_r, :] @ (h @ W)  runs as a
    dense matmul with the contraction (all N nodes) on the partition dim.
  - Node (row) sharding: core r owns nodes [r*LOC, (r+1)*LOC). Per GCN layer,
    each core computes hw_local = h_local @ W (features-on-partition layout),
    AllGathers hw to all cores, then does its A-shard matmul.
  - Attention is sequence-parallel: q stays local; h3 is AllGathered and every
    core builds K^T / V for all N nodes, computes its [LOC, N] score block per
    head, exp (no max subtraction - scores are tiny, validated on host), and
    accumulates o_unnorm^T = V^T exp^T and rowsums via TensorE. The row-mean of
    the attention output is reduced locally (softmax division folded into a
    weighted sum) and AllReduced, after which every core redundantly computes
    the small deformation head.
  - Bias algebra folded on host: k-proj bias drops (softmax shift invariance),
    v-proj bias and the 1/N mean fold into the out-proj, 1/sqrt(dh) folds into
    the q projection.
"""

import numpy as np
import ml_dtypes

import concourse.bacc as bacc
import concourse.bass as bass
import concourse.tile as tile
from concourse import bass_isa, mybir
from concourse.bass_utils import run_bass_kernel_spmd

F32 = mybir.dt.float32
BF16 = mybir.dt.bfloat16
AF = mybir.ActivationFunctionType
ALU = mybir.AluOpType
NPBF16 = ml_dtypes.bfloat16

P = 128


class Cfg:
    def __init__(self, n=4096, nc=8, din=16, d=256, h=4, adim=64, act="gelu"):
        self.N, self.NC, self.DIN, self.D, self.H, self.A = n, nc, din, d, h, adim
        self.act = act  # "tanh" only for simulator runs (sim has no Gelu)
        self.D2 = 2 * d
        self.DH = d // h
        self.LOC = n // nc
        self.KT = n // P          # node k-tiles
        self.FT = d // P          # feature tiles (2)
        self.LT = self.LOC // P   # local node tiles
        self.RG = [list(range(nc))]
        assert self.DH == 64 and self.FT == 2 and h == 4


# ----------------------------------------------------------------- host side

def _build_a_norm_t_shards(edge_index, cfg):
    """Per-core transposed shards of the normalized adjacency, bf16."""
    n = cfg.N
    src = np.asarray(edge_index[0]).astype(np.int64)
    dst = np.asarray(edge_index[1]).astype(np.int64)
    deg = np.bincount(dst, minlength=n).astype(np.float64) + 1.0
    dinv = 1.0 / np.sqrt(deg)
    amat = np.zeros((n, n), np.float32)
    np.add.at(amat, (dst, src), 1.0)
    amat[np.arange(n), np.arange(n)] += 1.0
    anorm = (dinv[:, None] * amat * dinv[None, :]).astype(np.float32)
    at = np.ascontiguousarray(anorm.T)  # [n src, n dst-rows]
    loc = cfg.LOC
    return [np.ascontiguousarray(at[:, r * loc:(r + 1) * loc]).astype(NPBF16)
            for r in range(cfg.NC)]


def _kcols(v):
    """[K] vector -> [P, K//P] column-chunk layout (partition = feature%128)."""
    v = np.asarray(v, np.float32)
    return np.ascontiguousarray(v.reshape(-1, P).T)


def preprocess(inputs, cfg):
    """FULL inputs -> list of per-core input dicts."""
    f32 = lambda a: np.ascontiguousarray(np.asarray(a, np.float32))
    bf = lambda a: np.ascontiguousarray(np.asarray(a, np.float32)).astype(NPBF16)

    x = f32(inputs["x"])
    wi = f32(inputs["in_proj_w"])          # [3D, D]
    bi = f32(inputs["in_proj_b"])          # [3D]
    wo = f32(inputs["out_proj_w"])         # [D, D]
    bo = f32(inputs["out_proj_b"])
    d = cfg.D

    wi_t = wi.T.copy()                     # [D, 3D]
    scale = 1.0 / np.sqrt(cfg.DH)
    wi_t[:, :d] *= scale                   # fold 1/sqrt(dh) into q proj
    bq = bi[:d] * scale
    bv = bi[2 * d:]
    bo_eff = bv @ wo.T + bo                # fold v bias through out proj
    wo_t_mean = wo.T.copy() / float(cfg.N)  # fold 1/N mean into out proj

    shared = {
        "xt": bf(x.T),                                   # [DIN, N]
        "w1": bf(inputs["W1"]), "w2": bf(inputs["W2"]), "w3": bf(inputs["W3"]),
        "b1k": _kcols(inputs["b1"]), "b2k": _kcols(inputs["b2"]),
        "b3k": _kcols(inputs["b3"]),
        "wit": bf(wi_t),                                 # [D, 3D]
        "biqk": _kcols(bq),                              # [P, FT]
        "wot": f32(wo_t_mean),                           # [D, D]
        "bok": _kcols(bo_eff),                           # [P, FT]
        "wd": f32(inputs["Wd"]),                         # [D, D2]
        "bdk": _kcols(inputs["bd"]),                     # [P, 4]
        "lngk": _kcols(inputs["ln_g"]), "lnbk": _kcols(inputs["ln_b"]),
        "wh": f32(inputs["Wh"]),                         # [D2, A]
        "bh": f32(inputs["bh"]).reshape(1, cfg.A),
    }
    a_shards = _build_a_norm_t_shards(inputs["edge_index"], cfg)
    return [dict(shared, at=a_shards[r]) for r in range(cfg.NC)]


# --------------------------------------------------------------- device side

def build_program(cfg):
    nc = bacc.Bacc("TRN2", target_bir_lowering=False, debug=False,
                   num_devices=cfg.NC)
    N, D, DIN, LOC, KT, FT, LT, D2, A = (cfg.N, cfg.D, cfg.DIN, cfg.LOC,
                                         cfg.KT, cfg.FT, cfg.LT, cfg.D2, cfg.A)
    RG = cfg.RG

    din = {}
    def ein(name, shape, dt):
        din[name] = nc.dram_tensor(name, list(shape), dt, kind="ExternalInput").ap()
        return din[name]

    at_d = ein("at", (N, LOC), BF16)
    xt_d = ein("xt", (DIN, N), BF16)
    w1_d = ein("w1", (DIN, D), BF16)
    w2_d = ein("w2", (D, D), BF16)
    w3_d = ein("w3", (D, D), BF16)
    b1k_d = ein("b1k", (P, FT), F32)
    b2k_d = ein("b2k", (P, FT), F32)
    b3k_d = ein("b3k", (P, FT), F32)
    wit_d = ein("wit", (D, 3 * D), BF16)
    biqk_d = ein("biqk", (P, FT), F32)
    wot_d = ein("wot", (D, D), F32)
    bok_d = ein("bok", (P, FT), F32)
    wd_d = ein("wd", (D, D2), F32)
    bdk_d = ein("bdk", (P, 4), F32)
    lngk_d = ein("lngk", (P, 4), F32)
    lnbk_d = ein("lnbk", (P, 4), F32)
    wh_d = ein("wh", (D2, A), F32)
    bh_d = ein("bh", (1, A), F32)

    hg_out = nc.dram_tensor("h_global", [1, D], F32, kind="ExternalOutput").ap()
    def_out = nc.dram_tensor("deformation", [1, A], F32, kind="ExternalOutput").ap()

    with tile.TileContext(nc) as tc:
        _emit(tc, cfg, din, hg_out, def_out)
    nc.compile()
    return nc


def _ktile_view(dram_ap, kt_count, m):
    """[K, M] dram -> [P, kt, M] view for k-tile loads."""
    return dram_ap.rearrange("(t p) m -> p t m", p=P)


def _emit(tc, cfg, din, hg_out, def_out):
    nc = tc.nc
    N, D, DIN, LOC, KT, FT, LT, D2, A, H = (cfg.N, cfg.D, cfg.DIN, cfg.LOC,
                                            cfg.KT, cfg.FT, cfg.LT, cfg.D2,
                                            cfg.A, cfg.H)
    RG = cfg.RG
    RKT = LOC // P                # k-tiles per rank block (4)

    import contextlib
    ctx = contextlib.ExitStack()
    consts = ctx.enter_context(tc.tile_pool(name="consts", bufs=1))
    a_pool = ctx.enter_context(tc.tile_pool(name="a_pool", bufs=1))
    hw_pool = ctx.enter_context(tc.tile_pool(name="hw_pool", bufs=2))
    ht_pool = ctx.enter_context(tc.tile_pool(name="ht_pool", bufs=2))
    sb_small = ctx.enter_context(tc.tile_pool(name="sb_small", bufs=3))
    kv_pool = ctx.enter_context(tc.tile_pool(name="kv_pool", bufs=1))
    h3f_pool = ctx.enter_context(tc.tile_pool(name="h3f_pool", bufs=3))
    exp_pool = ctx.enter_context(tc.tile_pool(name="exp_pool", bufs=6))
    head_pool = ctx.enter_context(tc.tile_pool(name="head_pool", bufs=1))
    dram = ctx.enter_context(tc.tile_pool(name="dram", bufs=2, space="DRAM"))

    # ---------------- constants to SBUF
    xt_sb = consts.tile([DIN, N], BF16)
    nc.sync.dma_start(out=xt_sb, in_=din["xt"])
    w1_sb = consts.tile([DIN, D], BF16)
    nc.sync.dma_start(out=w1_sb, in_=din["w1"])

    def load_ktiles(name, kdim, m, dt, pool=consts):
        t = pool.tile([P, kdim // P, m], dt, name=name)
        nc.sync.dma_start(out=t, in_=din[name].rearrange("(t p) m -> p t m", p=P))
        return t

    w2_sb = load_ktiles("w2", D, D, BF16)
    w3_sb = load_ktiles("w3", D, D, BF16)
    wit_sb = load_ktiles("wit", D, 3 * D, BF16)
    wot_sb = load_ktiles("wot", D, D, F32)
    wd_sb = load_ktiles("wd", D, D2, F32)
    wh_sb = load_ktiles("wh", D2, A, F32)

    def load_small(name, shape, dt):
        t = consts.tile(list(shape), dt, name=name)
        nc.sync.dma_start(out=t, in_=din[name])
        return t

    b1k_sb = load_small("b1k", (P, FT), F32)
    b2k_sb = load_small("b2k", (P, FT), F32)
    b3k_sb = load_small("b3k", (P, FT), F32)
    biqk_sb = load_small("biqk", (P, FT), F32)
    bok_sb = load_small("bok", (P, FT), F32)
    bdk_sb = load_small("bdk", (P, 4), F32)
    lngk_sb = load_small("lngk", (P, 4), F32)
    lnbk_sb = load_small("lnbk", (P, 4), F32)
    bh_sb = load_small("bh", (1, A), F32)

    ones_sb = consts.tile([P, 1], BF16)
    nc.vector.memset(ones_sb, 1.0)
    ones_f32 = consts.tile([P, P], F32)
    nc.vector.memset(ones_f32, 1.0)
    eps_sb = consts.tile([P, 1], F32)
    nc.vector.memset(eps_sb, 1e-5)

    # ---------------- A^T shard: [P, KT, LOC], loaded in 8 rank-block chunks
    at_sb = a_pool.tile([P, KT, LOC], BF16)
    at_view = din["at"].rearrange("(t p) l -> p t l", p=P)   # [P, KT, LOC]
    for r in range(cfg.NC):
        nc.sync.dma_start(out=at_sb[:, r * RKT:(r + 1) * RKT, :],
                          in_=at_view[:, r * RKT:(r + 1) * RKT, :])

    # ---------------- phase 1: hw1_full = x @ W1 (replicated, K=DIN)
    with tc.tile_pool(name="ps_gen", bufs=3, space="PSUM") as ps_gen:
        hw_sb = hw_pool.tile([P, KT, D], BF16, tag="hw")
        for mt in range(KT):
            ps = ps_gen.tile([P, D], F32, tag="ps")
            nc.tensor.matmul(ps, lhsT=xt_sb[:, mt * P:(mt + 1) * P],
                             rhs=w1_sb, start=True, stop=True)
            nc.vector.tensor_copy(out=hw_sb[:, mt, :], in_=ps)

        def a_mult(hw, bk, func):
            """out_hT[P, FT, LOC] = func(A_shard @ hw_full + b)."""
            h_t = ht_pool.tile([P, FT, LOC], BF16, tag="hT")
            for mf in range(FT):
                ps = ps_gen.tile([P, LOC], F32, tag="ps")
                for kt in range(KT):
                    nc.tensor.matmul(ps, lhsT=hw[:, kt, mf * P:(mf + 1) * P],
                                     rhs=at_sb[:, kt, :],
                                     start=(kt == 0), stop=(kt == KT - 1))
                nc.scalar.activation(out=h_t[:, mf, :], in_=ps, func=func,
                                     bias=bk[:, mf:mf + 1], scale=1.0)
            return h_t

        def hw_next_ag(h_t, w_sb, lname):
            """AllGather hw = h_local @ W -> [P, KT, D] bf16 lhsT tiles."""
            bounce = dram.tile([LOC, D], BF16, tag="hw_bounce", name=f"hwb_{lname}")
            for nt in range(LT):
                ps = ps_gen.tile([P, D], F32, tag="ps")
                for kf in range(FT):
                    nc.tensor.matmul(ps, lhsT=h_t[:, kf, nt * P:(nt + 1) * P],
                                     rhs=w_sb[:, kf, :],
                                     start=(kf == 0), stop=(kf == FT - 1))
                tmp = sb_small.tile([P, D], BF16, tag="hwtmp")
                nc.vector.tensor_copy(out=tmp, in_=ps)
                nc.sync.dma_start(out=bounce[nt * P:(nt + 1) * P, :], in_=tmp)
            full = dram.tile([cfg.NC, LOC, D], BF16, addr_space="Shared",
                             tag="hw_full", name=f"hwf_{lname}")
            nc.gpsimd.collective_compute("AllGather", ALU.bypass,
                                         replica_groups=RG,
                                         ins=[bounce.opt()], outs=[full.opt()])
            hw = hw_pool.tile([P, KT, D], BF16, tag="hw", name=f"hw_{lname}")
            for r in range(cfg.NC):
                nc.sync.dma_start(
                    out=hw[:, r * RKT:(r + 1) * RKT, :],
                    in_=full[r].rearrange("(t p) d -> p t d", p=P))
            return hw

        gelu_f = AF.Gelu if cfg.act == "gelu" else AF.Tanh
        h1_t = a_mult(hw_sb, b1k_sb, gelu_f)
        hw2 = hw_next_ag(h1_t, w2_sb, "l2")
        h2_t = a_mult(hw2, b2k_sb, gelu_f)
        hw3 = hw_next_ag(h2_t, w3_sb, "l3")
        h3_t = a_mult(hw3, b3k_sb, AF.Identity)

        # -------- h3 AllGather (for K/V); q projection from local h3
        h3b = dram.tile([D, LOC], BF16, tag="h3b")
        for kf in range(FT):
            nc.sync.dma_start(out=h3b[kf * P:(kf + 1) * P, :], in_=h3_t[:, kf, :])
        h3f = dram.tile([cfg.NC, D, LOC], BF16, addr_space="Shared", tag="h3f")
        nc.gpsimd.collective_compute("AllGather", ALU.bypass, replica_groups=RG,
                                     ins=[h3b.opt()], outs=[h3f.opt()])

        q_t = ht_pool.tile([P, FT, LOC], BF16, tag="qT")
        for mf in range(FT):
            ps = ps_gen.tile([P, LOC], F32, tag="ps")
            for kf in range(FT):
                nc.tensor.matmul(ps, lhsT=wit_sb[:, kf, mf * P:(mf + 1) * P],
                                 rhs=h3_t[:, kf, :],
                                 start=(kf == 0), stop=(kf == FT - 1))
            nc.scalar.activation(out=q_t[:, mf, :], in_=ps, func=AF.Identity,
                                 bias=biqk_sb[:, mf:mf + 1], scale=1.0)

        # -------- K^T [P, FT, N] and V [P, KT, D] for all nodes
        kt_sb = kv_pool.tile([P, FT, N], BF16)
        v_sb = kv_pool.tile([P, KT, D], BF16)
        for r in range(cfg.NC):
            h3r = h3f_pool.tile([P, FT, LOC], BF16, tag="h3r")
            for kf in range(FT):
                nc.sync.dma_start(out=h3r[:, kf, :],
                                  in_=h3f[r, kf * P:(kf + 1) * P, :])
            for mf in range(FT):  # K^T block for this rank
                ps = ps_gen.tile([P, LOC], F32, tag="ps")
                for kf in range(FT):
                    nc.tensor.matmul(
                        ps, lhsT=wit_sb[:, kf, D + mf * P:D + (mf + 1) * P],
                        rhs=h3r[:, kf, :],
                        start=(kf == 0), stop=(kf == FT - 1))
                nc.vector.tensor_copy(out=kt_sb[:, mf, r * LOC:(r + 1) * LOC],
                                      in_=ps)
            for nt in range(RKT):  # V block (natural layout)
                ps = ps_gen.tile([P, D], F32, tag="ps")
                for kf in range(FT):
                    nc.tensor.matmul(
                        ps, lhsT=h3r[:, kf, nt * P:(nt + 1) * P],
                        rhs=wit_sb[:, kf, 2 * D:3 * D],
                        start=(kf == 0), stop=(kf == FT - 1))
                nc.vector.tensor_copy(out=v_sb[:, r * RKT + nt, :], in_=ps)

    # ---------------- attention main loop
    with tc.tile_pool(name="ps_acc", bufs=1, space="PSUM") as ps_acc:
        o_ps = [ps_acc.tile([P, LOC], F32, name=f"o_ps{i}") for i in range(2)]
        rs_ps = ps_acc.tile([P, LOC], F32, name="rs_ps")
        with tc.tile_pool(name="ps_s", bufs=4, space="PSUM") as ps_s:
            for kt in range(KT):
                for h in range(H):
                    pair, sub = h // 2, h % 2
                    sp = ps_s.tile([P, LOC], F32, tag="s")
                    nc.tensor.matmul(
                        sp,
                        lhsT=kt_sb[sub * 64:(sub + 1) * 64, pair,
                                   kt * P:(kt + 1) * P],
                        rhs=q_t[sub * 64:(sub + 1) * 64, pair, :],
                        start=True, stop=True)
                    e = exp_pool.tile([P, LOC], BF16, tag="e")
                    nc.scalar.activation(out=e, in_=sp, func=AF.Exp)
                    nc.tensor.matmul(
                        o_ps[pair][sub * 64:(sub + 1) * 64, :],
                        lhsT=v_sb[:, kt, h * 64:(h + 1) * 64], rhs=e,
                        start=(kt == 0), stop=(kt == KT - 1),
                        tile_position=(0, sub * 64), skip_group_check=True)
                    nc.tensor.matmul(
                        rs_ps[32 * h:32 * h + 1, :], lhsT=ones_sb, rhs=e,
                        start=(kt == 0), stop=(kt == KT - 1),
                        tile_position=(0, 32 * h), skip_group_check=True)

        # -------- local sum over q rows of o/rowsum -> m_partial [P, FT]
        # rowsum rows live at partitions 0/32/64/96; copy to SBUF, broadcast
        # across 64 partitions with a K=1 PE matmul, divide + free-reduce.
        with tc.tile_pool(name="ps_bc", bufs=2, space="PSUM") as ps_bc:
            rs_sb = head_pool.tile([P, LOC], F32)
            bc_sb = head_pool.tile([P, 2, LOC], F32)
            m_sb = head_pool.tile([P, FT], F32)
            junk = head_pool.tile([P, 2, LOC], F32)
            for h in range(H):
                pair, sub = h // 2, h % 2
                nc.vector.reciprocal(out=rs_sb[32 * h:32 * h + 1, :],
                                     in_=rs_ps[32 * h:32 * h + 1, :])
                bcp = ps_bc.tile([P, LOC], F32, tag="bc")
                nc.tensor.matmul(
                    bcp[sub * 64:(sub + 1) * 64, :],
                    lhsT=ones_f32[32 * h:32 * h + 1, :64],
                    rhs=rs_sb[32 * h:32 * h + 1, :],
                    start=True, stop=True, tile_position=(32 * h, sub * 64),
                    skip_group_check=True)
                nc.scalar.copy(out=bc_sb[sub * 64:(sub + 1) * 64, pair, :],
                               in_=bcp[sub * 64:(sub + 1) * 64, :])
                nc.vector.tensor_mul(
                    out=junk[sub * 64:(sub + 1) * 64, pair, :],
                    in0=o_ps[pair][sub * 64:(sub + 1) * 64, :],
                    in1=bc_sb[sub * 64:(sub + 1) * 64, pair, :])
                nc.vector.tensor_reduce(
                    out=m_sb[sub * 64:(sub + 1) * 64, pair:pair + 1],
                    in_=junk[sub * 64:(sub + 1) * 64, pair, :],
                    axis=mybir.AxisListType.X, op=ALU.add)

        mb = dram.tile([P, FT], F32, tag="mb")
        nc.sync.dma_start(out=mb, in_=m_sb)
        mf_sh = dram.tile([P, FT], F32, addr_space="Shared", tag="mf_sh")
        nc.gpsimd.collective_compute("AllReduce", ALU.add, replica_groups=RG,
                                     ins=[mb.opt()], outs=[mf_sh.opt()])
        msum_sb = head_pool.tile([P, FT], F32)
        nc.sync.dma_start(out=msum_sb, in_=mf_sh)

    # ---------------- deformation head (replicated, tiny)
    with tc.tile_pool(name="ps_head", bufs=1, space="PSUM") as ps_head:
        # h_global^T (k-layout [P, FT]) = wot^T @ msum ; + bok
        ps_hg = ps_head.tile([P, FT], F32)
        for mf in range(FT):
            for c in range(FT):
                nc.tensor.matmul(ps_hg[:, mf:mf + 1],
                                 lhsT=wot_sb[:, c, mf * P:(mf + 1) * P],
                                 rhs=msum_sb[:, c:c + 1],
                                 start=(c == 0), stop=(c == FT - 1),
                                 skip_group_check=True)
        hgk_sb = head_pool.tile([P, FT], F32)
        nc.vector.tensor_add(out=hgk_sb, in0=ps_hg, in1=bok_sb)
        nc.sync.dma_start(out=hg_out.rearrange("a (c p) -> p (c a)", p=P),
                          in_=hgk_sb)

        # t1 (k-layout [P, 4]) = wd^T @ hgk + bd
        ps_t = ps_head.tile([P, 4], F32)
        for mt in range(4):
            for c in range(FT):
                nc.tensor.matmul(ps_t[:, mt:mt + 1],
                                 lhsT=wd_sb[:, c, mt * P:(mt + 1) * P],
                                 rhs=hgk_sb[:, c:c + 1],
                                 start=(c == 0), stop=(c == FT - 1),
                                 skip_group_check=True)
        # t1 and t1^2 side by side -> partition-sum via ones-matmul
        t1_sb = head_pool.tile([P, 8], F32)
        nc.vector.tensor_add(out=t1_sb[:, 0:4], in0=ps_t, in1=bdk_sb)
        nc.scalar.activation(out=t1_sb[:, 4:8], in_=t1_sb[:, 0:4],
                             func=AF.Square)

        # LayerNorm over all 512 values (spread across partitions x 4)
        ps_sum = ps_head.tile([1, 8], F32)
        nc.tensor.matmul(ps_sum, lhsT=ones_f32[:, 0:1], rhs=t1_sb,
                         start=True, stop=True)
        sums_sb = head_pool.tile([1, 2], F32)
        nc.vector.tensor_reduce(out=sums_sb[:, 0:1], in_=ps_sum[0:1, 0:4],
                                axis=mybir.AxisListType.X, op=ALU.add)
        nc.vector.tensor_reduce(out=sums_sb[:, 1:2], in_=ps_sum[0:1, 4:8],
                                axis=mybir.AxisListType.X, op=ALU.add)
        ps_bc2 = ps_head.tile([P, 2], F32)
        nc.tensor.matmul(ps_bc2, lhsT=ones_f32[0:1, :], rhs=sums_sb,
                         start=True, stop=True)
        musq_sb = head_pool.tile([P, 2], F32)
        nc.scalar.activation(out=musq_sb, in_=ps_bc2, func=AF.Identity,
                             scale=1.0 / D2)
        mu_sb = musq_sb[:, 0:1]
        msq_sb = musq_sb[:, 1:2]
        var_sb = head_pool.tile([P, 1], F32)
        nc.vector.tensor_mul(out=var_sb, in0=mu_sb, in1=mu_sb)
        nc.vector.tensor_sub(out=var_sb, in0=msq_sb, in1=var_sb)
        nc.scalar.activation(out=var_sb, in_=var_sb, func=AF.Sqrt,
                             bias=eps_sb, scale=1.0)
        rstd_sb = head_pool.tile([P, 1], F32)
        nc.vector.reciprocal(out=rstd_sb, in_=var_sb)
        tn_sb = head_pool.tile([P, 4], F32)
        nc.vector.tensor_scalar(out=tn_sb, in0=t1_sb[:, 0:4], scalar1=mu_sb,
                                scalar2=rstd_sb, op0=ALU.subtract, op1=ALU.mult)
        nc.vector.tensor_mul(out=tn_sb, in0=tn_sb, in1=lngk_sb)
        nc.vector.tensor_add(out=tn_sb, in0=tn_sb, in1=lnbk_sb)
        nc.scalar.activation(out=tn_sb, in_=tn_sb,
                             func=AF.Gelu if cfg.act == "gelu" else AF.Tanh)

        # u = t @ Wh + bh  -> [1, A]
        ps_u = ps_head.tile([1, A], F32)
        for j in range(4):
            nc.tensor.matmul(ps_u, lhsT=tn_sb[:, j:j + 1], rhs=wh_sb[:, j, :],
                             start=(j == 0), stop=(j == 3))
        u_sb = head_pool.tile([1, A], F32)
        nc.vector.tensor_add(out=u_sb, in0=ps_u, in1=bh_sb)

        # expmap0: u * tanh(|u|)/|u|, |u| clamped at 1e-5
        usq_sb = head_pool.tile([1, A], F32)
        ss_sb = head_pool.tile([1, 1], F32)
        nc.scalar.activation(out=usq_sb, in_=u_sb, func=AF.Square,
                             accum_out=ss_sb)
        nn_sb = head_pool.tile([1, 1], F32)
        nc.scalar.activation(out=nn_sb, in_=ss_sb, func=AF.Sqrt,
                             bias=0.0, scale=1.0)
        nc.vector.tensor_scalar_max(out=nn_sb, in0=nn_sb, scalar1=1e-5)
        th_sb = head_pool.tile([1, 1], F32)
        nc.scalar.activation(out=th_sb, in_=nn_sb, func=AF.Tanh)
        ninv_sb = head_pool.tile([1, 1], F32)
        nc.vector.reciprocal(out=ninv_sb, in_=nn_sb)
        fac_sb = head_pool.tile([1, 1], F32)
        nc.vector.tensor_mul(out=fac_sb, in0=th_sb, in1=ninv_sb)
        dout_sb = head_pool.tile([1, A], F32)
        nc.vector.tensor_scalar_mul(out=dout_sb, in0=u_sb, scalar1=fac_sb)
        nc.sync.dma_start(out=def_out, in_=dout_sb)

    ctx.close()


# ----------------------------------------------------------------- entry

_CACHE = {}


def _get_program(cfg):
    key = (cfg.N, cfg.NC)
    if key not in _CACHE:
        _CACHE[key] = build_program(cfg)
    return _CACHE[key]


def kernel(**inputs):
    cfg = Cfg()
    nc = _get_program(cfg)
    in_maps = preprocess(inputs, cfg)
    res = run_bass_kernel_spmd(nc, in_maps, core_ids=list(range(cfg.NC)))
    out = res.results[0]
    return (np.asarray(out["deformation"], np.float32),
            np.asarray(out["h_global"], np.float32))


# revision 19
# speedup vs baseline: 1.2788x; 1.2788x over previous
"""Trainium2 Bass kernel for nn_CYActorNetwork (GCN x3 -> MHA -> global mean -> MLP head).

Strategy (8 NeuronCores, SPMD):
  - Graph preprocessing on host: edge_index -> dense normalized adjacency
    A_norm = D^-1/2 (Adj + I) D^-1/2  (exactly matches PyG GCNConv with
    self-loops + symmetric normalization, including duplicate-edge
    multiplicity). Each core gets the transposed column-slice A_norm.T[:, rows_r]
    so the per-layer aggregation  out_r = A_norm[rows_r, :] @ (h @ W)  runs as a
    dense matmul with the contraction (all N nodes) on the partition dim.
  - Node (row) sharding: core r owns nodes [r*LOC, (r+1)*LOC). Per GCN layer,
    each core computes hw_local = h_local @ W (features-on-partition layout),
    AllGathers hw to all cores, then does its A-shard matmul.
  - Attention is sequence-parallel: q stays local; h3 is AllGathered and every
    core builds K^T / V for all N nodes, computes its [LOC, N] score block per
    head, exp (no max subtraction - scores are tiny, validated on host), and
    accumulates o_unnorm^T = V^T exp^T and rowsums via TensorE. The row-mean of
    the attention output is reduced locally (softmax division folded into a
    weighted sum) and AllReduced, after which every core redundantly computes
    the small deformation head.
  - Bias algebra folded on host: k-proj bias drops (softmax shift invariance),
    v-proj bias and the 1/N mean fold into the out-proj, 1/sqrt(dh) folds into
    the q projection.
"""

import numpy as np
import ml_dtypes

import concourse.bacc as bacc
import concourse.bass as bass
import concourse.tile as tile
from concourse import bass_isa, mybir
from concourse.bass_utils import run_bass_kernel_spmd

F32 = mybir.dt.float32
BF16 = mybir.dt.bfloat16
AF = mybir.ActivationFunctionType
ALU = mybir.AluOpType
NPBF16 = ml_dtypes.bfloat16

P = 128


class Cfg:
    def __init__(self, n=4096, nc=8, din=16, d=256, h=4, adim=64, act="gelu"):
        self.N, self.NC, self.DIN, self.D, self.H, self.A = n, nc, din, d, h, adim
        self.act = act  # "tanh" only for simulator runs (sim has no Gelu)
        self.D2 = 2 * d
        self.DH = d // h
        self.LOC = n // nc
        self.KT = n // P          # node k-tiles
        self.FT = d // P          # feature tiles (2)
        self.LT = self.LOC // P   # local node tiles
        self.RG = [list(range(nc))]
        assert self.DH == 64 and self.FT == 2 and h == 4


# ----------------------------------------------------------------- host side

def _build_a_norm_t_shards(edge_index, cfg):
    """Per-core transposed shards of the normalized adjacency, bf16."""
    n = cfg.N
    src = np.asarray(edge_index[0]).astype(np.int64)
    dst = np.asarray(edge_index[1]).astype(np.int64)
    deg = np.bincount(dst, minlength=n).astype(np.float64) + 1.0
    dinv = 1.0 / np.sqrt(deg)
    amat = np.zeros((n, n), np.float32)
    np.add.at(amat, (dst, src), 1.0)
    amat[np.arange(n), np.arange(n)] += 1.0
    anorm = (dinv[:, None] * amat * dinv[None, :]).astype(np.float32)
    at = np.ascontiguousarray(anorm.T)  # [n src, n dst-rows]
    loc = cfg.LOC
    return [np.ascontiguousarray(at[:, r * loc:(r + 1) * loc]).astype(NPBF16)
            for r in range(cfg.NC)]


def _kcols(v):
    """[K] vector -> [P, K//P] column-chunk layout (partition = feature%128)."""
    v = np.asarray(v, np.float32)
    return np.ascontiguousarray(v.reshape(-1, P).T)


def preprocess(inputs, cfg):
    """FULL inputs -> list of per-core input dicts."""
    f32 = lambda a: np.ascontiguousarray(np.asarray(a, np.float32))
    bf = lambda a: np.ascontiguousarray(np.asarray(a, np.float32)).astype(NPBF16)

    x = f32(inputs["x"])
    wi = f32(inputs["in_proj_w"])          # [3D, D]
    bi = f32(inputs["in_proj_b"])          # [3D]
    wo = f32(inputs["out_proj_w"])         # [D, D]
    bo = f32(inputs["out_proj_b"])
    d = cfg.D

    wi_t = wi.T.copy()                     # [D, 3D]
    scale = 1.0 / np.sqrt(cfg.DH)
    wi_t[:, :d] *= scale                   # fold 1/sqrt(dh) into q proj
    bq = bi[:d] * scale
    bv = bi[2 * d:]
    bo_eff = bv @ wo.T + bo                # fold v bias through out proj
    wo_t_mean = wo.T.copy() / float(cfg.N)  # fold 1/N mean into out proj

    shared = {
        "xt": bf(x.T),                                   # [DIN, N]
        "w1": bf(inputs["W1"]), "w2": bf(inputs["W2"]), "w3": bf(inputs["W3"]),
        "b1k": _kcols(inputs["b1"]), "b2k": _kcols(inputs["b2"]),
        "b3k": _kcols(inputs["b3"]),
        "wit": bf(wi_t),                                 # [D, 3D]
        "biqk": _kcols(bq),                              # [P, FT]
        "wot": f32(wo_t_mean),                           # [D, D]
        "bok": _kcols(bo_eff),                           # [P, FT]
        "wd": f32(inputs["Wd"]),                         # [D, D2]
        "bdk": _kcols(inputs["bd"]),                     # [P, 4]
        "lngk": _kcols(inputs["ln_g"]), "lnbk": _kcols(inputs["ln_b"]),
        "wh": f32(inputs["Wh"]),                         # [D2, A]
        "bh": f32(inputs["bh"]).reshape(1, cfg.A),
    }
    a_shards = _build_a_norm_t_shards(inputs["edge_index"], cfg)
    return [dict(shared, at=a_shards[r]) for r in range(cfg.NC)]


# --------------------------------------------------------------- device side

def build_program(cfg):
    nc = bacc.Bacc("TRN2", target_bir_lowering=False, debug=False,
                   num_devices=cfg.NC)
    N, D, DIN, LOC, KT, FT, LT, D2, A = (cfg.N, cfg.D, cfg.DIN, cfg.LOC,
                                         cfg.KT, cfg.FT, cfg.LT, cfg.D2, cfg.A)
    RG = cfg.RG

    din = {}
    def ein(name, shape, dt):
        din[name] = nc.dram_tensor(name, list(shape), dt, kind="ExternalInput").ap()
        return din[name]

    at_d = ein("at", (N, LOC), BF16)
    xt_d = ein("xt", (DIN, N), BF16)
    w1_d = ein("w1", (DIN, D), BF16)
    w2_d = ein("w2", (D, D), BF16)
    w3_d = ein("w3", (D, D), BF16)
    b1k_d = ein("b1k", (P, FT), F32)
    b2k_d = ein("b2k", (P, FT), F32)
    b3k_d = ein("b3k", (P, FT), F32)
    wit_d = ein("wit", (D, 3 * D), BF16)
    biqk_d = ein("biqk", (P, FT), F32)
    wot_d = ein("wot", (D, D), F32)
    bok_d = ein("bok", (P, FT), F32)
    wd_d = ein("wd", (D, D2), F32)
    bdk_d = ein("bdk", (P, 4), F32)
    lngk_d = ein("lngk", (P, 4), F32)
    lnbk_d = ein("lnbk", (P, 4), F32)
    wh_d = ein("wh", (D2, A), F32)
    bh_d = ein("bh", (1, A), F32)

    hg_out = nc.dram_tensor("h_global", [1, D], F32, kind="ExternalOutput").ap()
    def_out = nc.dram_tensor("deformation", [1, A], F32, kind="ExternalOutput").ap()

    with tile.TileContext(nc) as tc:
        _emit(tc, cfg, din, hg_out, def_out)
    nc.compile()
    return nc


def _ktile_view(dram_ap, kt_count, m):
    """[K, M] dram -> [P, kt, M] view for k-tile loads."""
    return dram_ap.rearrange("(t p) m -> p t m", p=P)


def _emit(tc, cfg, din, hg_out, def_out):
    nc = tc.nc
    N, D, DIN, LOC, KT, FT, LT, D2, A, H = (cfg.N, cfg.D, cfg.DIN, cfg.LOC,
                                            cfg.KT, cfg.FT, cfg.LT, cfg.D2,
                                            cfg.A, cfg.H)
    RG = cfg.RG
    RKT = LOC // P                # k-tiles per rank block (4)

    import contextlib
    ctx = contextlib.ExitStack()
    consts = ctx.enter_context(tc.tile_pool(name="consts", bufs=1))
    a_pool = ctx.enter_context(tc.tile_pool(name="a_pool", bufs=1))
    hw_pool = ctx.enter_context(tc.tile_pool(name="hw_pool", bufs=2))
    ht_pool = ctx.enter_context(tc.tile_pool(name="ht_pool", bufs=2))
    sb_small = ctx.enter_context(tc.tile_pool(name="sb_small", bufs=3))
    kv_pool = ctx.enter_context(tc.tile_pool(name="kv_pool", bufs=1))
    h3f_pool = ctx.enter_context(tc.tile_pool(name="h3f_pool", bufs=3))
    exp_pool = ctx.enter_context(tc.tile_pool(name="exp_pool", bufs=6))
    head_pool = ctx.enter_context(tc.tile_pool(name="head_pool", bufs=1))
    dram = ctx.enter_context(tc.tile_pool(name="dram", bufs=2, space="DRAM"))

    # ---------------- constants to SBUF
    xt_sb = consts.tile([DIN, N], BF16)
    nc.sync.dma_start(out=xt_sb, in_=din["xt"])
    w1_sb = consts.tile([DIN, D], BF16)
    nc.sync.dma_start(out=w1_sb, in_=din["w1"])

    def load_ktiles(name, kdim, m, dt, pool=consts):
        t = pool.tile([P, kdim // P, m], dt, name=name)
        nc.sync.dma_start(out=t, in_=din[name].rearrange("(t p) m -> p t m", p=P))
        return t

    w2_sb = load_ktiles("w2", D, D, BF16)
    w3_sb = load_ktiles("w3", D, D, BF16)
    wit_sb = load_ktiles("wit", D, 3 * D, BF16)
    wot_sb = load_ktiles("wot", D, D, F32)
    wd_sb = load_ktiles("wd", D, D2, F32)
    wh_sb = load_ktiles("wh", D2, A, F32)

    def load_small(name, shape, dt):
        t = consts.tile(list(shape), dt, name=name)
        nc.sync.dma_start(out=t, in_=din[name])
        return t

    b1k_sb = load_small("b1k", (P, FT), F32)
    b2k_sb = load_small("b2k", (P, FT), F32)
    b3k_sb = load_small("b3k", (P, FT), F32)
    biqk_sb = load_small("biqk", (P, FT), F32)
    bok_sb = load_small("bok", (P, FT), F32)
    bdk_sb = load_small("bdk", (P, 4), F32)
    lngk_sb = load_small("lngk", (P, 4), F32)
    lnbk_sb = load_small("lnbk", (P, 4), F32)
    bh_sb = load_small("bh", (1, A), F32)

    ones_sb = consts.tile([P, 1], BF16)
    nc.vector.memset(ones_sb, 1.0)
    ones_f32 = consts.tile([P, P], F32)
    nc.vector.memset(ones_f32, 1.0)
    eps_sb = consts.tile([P, 1], F32)
    nc.vector.memset(eps_sb, 1e-5)

    # ---------------- A^T shard: [P, KT, LOC], loaded in 8 rank-block chunks
    at_sb = a_pool.tile([P, KT, LOC], BF16)
    at_view = din["at"].rearrange("(t p) l -> p t l", p=P)   # [P, KT, LOC]
    for r in range(cfg.NC):
        nc.sync.dma_start(out=at_sb[:, r * RKT:(r + 1) * RKT, :],
                          in_=at_view[:, r * RKT:(r + 1) * RKT, :])

    # ---------------- phase 1: hw1_full = x @ W1 (replicated, K=DIN)
    with tc.tile_pool(name="ps_gen", bufs=3, space="PSUM") as ps_gen:
        hw_sb = hw_pool.tile([P, KT, D], BF16, tag="hw")
        for mt in range(KT):
            ps = ps_gen.tile([P, D], F32, tag="ps")
            nc.tensor.matmul(ps, lhsT=xt_sb[:, mt * P:(mt + 1) * P],
                             rhs=w1_sb, start=True, stop=True)
            nc.vector.tensor_copy(out=hw_sb[:, mt, :], in_=ps)

        def a_mult(hw, bk, func):
            """out_hT[P, FT, LOC] = func(A_shard @ hw_full + b)."""
            h_t = ht_pool.tile([P, FT, LOC], BF16, tag="hT")
            for mf in range(FT):
                ps = ps_gen.tile([P, LOC], F32, tag="ps")
                for kt in range(KT):
                    nc.tensor.matmul(ps, lhsT=hw[:, kt, mf * P:(mf + 1) * P],
                                     rhs=at_sb[:, kt, :],
                                     start=(kt == 0), stop=(kt == KT - 1))
                nc.scalar.activation(out=h_t[:, mf, :], in_=ps, func=func,
                                     bias=bk[:, mf:mf + 1], scale=1.0)
            return h_t

        def hw_next_ag(h_t, w_sb, lname):
            """AllGather hw = h_local @ W -> [P, KT, D] bf16 lhsT tiles."""
            bounce = dram.tile([LOC, D], BF16, tag="hw_bounce", name=f"hwb_{lname}")
            for nt in range(LT):
                ps = ps_gen.tile([P, D], F32, tag="ps")
                for kf in range(FT):
                    nc.tensor.matmul(ps, lhsT=h_t[:, kf, nt * P:(nt + 1) * P],
                                     rhs=w_sb[:, kf, :],
                                     start=(kf == 0), stop=(kf == FT - 1))
                tmp = sb_small.tile([P, D], BF16, tag="hwtmp")
                nc.vector.tensor_copy(out=tmp, in_=ps)
                nc.sync.dma_start(out=bounce[nt * P:(nt + 1) * P, :], in_=tmp)
            full = dram.tile([cfg.NC, LOC, D], BF16, addr_space="Shared",
                             tag="hw_full", name=f"hwf_{lname}")
            nc.gpsimd.collective_compute("AllGather", ALU.bypass,
                                         replica_groups=RG,
                                         ins=[bounce.opt()], outs=[full.opt()])
            hw = hw_pool.tile([P, KT, D], BF16, tag="hw", name=f"hw_{lname}")
            for r in range(cfg.NC):
                nc.sync.dma_start(
                    out=hw[:, r * RKT:(r + 1) * RKT, :],
                    in_=full[r].rearrange("(t p) d -> p t d", p=P))
            return hw

        gelu_f = AF.Gelu if cfg.act == "gelu" else AF.Tanh
        h1_t = a_mult(hw_sb, b1k_sb, gelu_f)
        hw2 = hw_next_ag(h1_t, w2_sb, "l2")
        h2_t = a_mult(hw2, b2k_sb, gelu_f)
        hw3 = hw_next_ag(h2_t, w3_sb, "l3")
        h3_t = a_mult(hw3, b3k_sb, AF.Identity)

        # -------- h3 AllGather (for K/V); q projection from local h3
        h3b = dram.tile([D, LOC], BF16, tag="h3b")
        for kf in range(FT):
            nc.sync.dma_start(out=h3b[kf * P:(kf + 1) * P, :], in_=h3_t[:, kf, :])
        h3f = dram.tile([cfg.NC, D, LOC], BF16, addr_space="Shared", tag="h3f")
        nc.gpsimd.collective_compute("AllGather", ALU.bypass, replica_groups=RG,
                                     ins=[h3b.opt()], outs=[h3f.opt()])

        q_t = ht_pool.tile([P, FT, LOC], BF16, tag="qT")
        for mf in range(FT):
            ps = ps_gen.tile([P, LOC], F32, tag="ps")
            for kf in range(FT):
                nc.tensor.matmul(ps, lhsT=wit_sb[:, kf, mf * P:(mf + 1) * P],
                                 rhs=h3_t[:, kf, :],
                                 start=(kf == 0), stop=(kf == FT - 1))
            nc.scalar.activation(out=q_t[:, mf, :], in_=ps, func=AF.Identity,
                                 bias=biqk_sb[:, mf:mf + 1], scale=1.0)

        # -------- K^T [P, FT, N] and V|ones [P, KT, H, DH+1] for all nodes
        # (65th column per head is 1.0 so the o-matmul's 65th output row
        # accumulates the softmax denominator for free)
        kt_sb = kv_pool.tile([P, FT, N], BF16)
        v_sb = kv_pool.tile([P, KT, H, cfg.DH + 1], BF16)
        nc.vector.memset(v_sb[:, :, :, cfg.DH:cfg.DH + 1], 1.0)
        for r in range(cfg.NC):
            h3r = h3f_pool.tile([P, FT, LOC], BF16, tag="h3r")
            for kf in range(FT):
                nc.sync.dma_start(out=h3r[:, kf, :],
                                  in_=h3f[r, kf * P:(kf + 1) * P, :])
            for mf in range(FT):  # K^T block for this rank
                ps = ps_gen.tile([P, LOC], F32, tag="ps")
                for kf in range(FT):
                    nc.tensor.matmul(
                        ps, lhsT=wit_sb[:, kf, D + mf * P:D + (mf + 1) * P],
                        rhs=h3r[:, kf, :],
                        start=(kf == 0), stop=(kf == FT - 1))
                nc.vector.tensor_copy(out=kt_sb[:, mf, r * LOC:(r + 1) * LOC],
                                      in_=ps)
            for nt in range(RKT):  # V block (natural layout)
                ps = ps_gen.tile([P, D], F32, tag="ps")
                for kf in range(FT):
                    nc.tensor.matmul(
                        ps, lhsT=h3r[:, kf, nt * P:(nt + 1) * P],
                        rhs=wit_sb[:, kf, 2 * D:3 * D],
                        start=(kf == 0), stop=(kf == FT - 1))
                nc.vector.tensor_copy(
                    out=v_sb[:, r * RKT + nt, :, 0:cfg.DH],
                    in_=ps.rearrange("p (h c) -> p h c", h=H))

    # ---------------- attention main loop
    # per head: o_acc[0:64] = V^T exp(S^T) accumulated over node k-tiles,
    # o_acc[64] = rowsum (ones column of V). Heads pair-share the exp pass.
    with tc.tile_pool(name="ps_acc", bufs=1, space="PSUM") as ps_acc:
        o_ps = [ps_acc.tile([P, LOC], F32, name=f"o_ps{i}") for i in range(H)]
        with tc.tile_pool(name="ps_s", bufs=2, space="PSUM") as ps_s:
            for kt in range(KT):
                for pair in range(2):
                    sp = ps_s.tile([P, 2, LOC], F32, tag="s")
                    for sub in range(2):
                        nc.tensor.matmul(
                            sp[:, sub, :],
                            lhsT=kt_sb[sub * 64:(sub + 1) * 64, pair,
                                       kt * P:(kt + 1) * P],
                            rhs=q_t[sub * 64:(sub + 1) * 64, pair, :],
                            start=True, stop=True, skip_group_check=True)
                    e = exp_pool.tile([P, 2, LOC], BF16, tag="e")
                    nc.scalar.activation(out=e, in_=sp, func=AF.Exp)
                    for sub in range(2):
                        h = 2 * pair + sub
                        nc.tensor.matmul(
                            o_ps[h][0:65, :],
                            lhsT=v_sb[:, kt, h, :], rhs=e[:, sub, :],
                            start=(kt == 0), stop=(kt == KT - 1),
                            skip_group_check=True)

        # -------- local sum over q rows of o/rowsum -> m_partial, via DRAM
        # bounce (DMA moves partitions; engines are partition-locked)
        with tc.tile_pool(name="ps_bc", bufs=2, space="PSUM") as ps_bc:
            inv_sb = head_pool.tile([P, H, LOC], F32)
            bc_sb = head_pool.tile([P, H, LOC], F32)
            mh_sb = head_pool.tile([P, H], F32)
            junk = head_pool.tile([P, H, LOC], F32)
            mb = dram.tile([P, FT], F32, tag="mb")
            for h in range(H):
                nc.vector.reciprocal(out=inv_sb[64:65, h, :],
                                     in_=o_ps[h][64:65, :])
                bcp = ps_bc.tile([P, LOC], F32, tag="bc")
                nc.tensor.matmul(
                    bcp[0:64, :], lhsT=ones_f32[64:65, :64],
                    rhs=inv_sb[64:65, h, :],
                    start=True, stop=True, tile_position=(64, 0),
                    skip_group_check=True)
                nc.scalar.copy(out=bc_sb[0:64, h, :], in_=bcp[0:64, :])
                nc.vector.tensor_mul(out=junk[0:64, h, :],
                                     in0=o_ps[h][0:64, :],
                                     in1=bc_sb[0:64, h, :])
                nc.vector.tensor_reduce(
                    out=mh_sb[0:64, h:h + 1], in_=junk[0:64, h, :],
                    axis=mybir.AxisListType.X, op=ALU.add)
                nc.sync.dma_start(
                    out=mb[(h % 2) * 64:(h % 2) * 64 + 64, h // 2:h // 2 + 1],
                    in_=mh_sb[0:64, h:h + 1])
        mf_sh = dram.tile([P, FT], F32, addr_space="Shared", tag="mf_sh")
        nc.gpsimd.collective_compute("AllReduce", ALU.add, replica_groups=RG,
                                     ins=[mb.opt()], outs=[mf_sh.opt()])
        msum_sb = head_pool.tile([P, FT], F32)
        nc.sync.dma_start(out=msum_sb, in_=mf_sh)

    # ---------------- deformation head (replicated, tiny)
    with tc.tile_pool(name="ps_head", bufs=1, space="PSUM") as ps_head:
        # h_global^T (k-layout [P, FT]) = wot^T @ msum ; + bok
        ps_hg = ps_head.tile([P, FT], F32)
        for mf in range(FT):
            for c in range(FT):
                nc.tensor.matmul(ps_hg[:, mf:mf + 1],
                                 lhsT=wot_sb[:, c, mf * P:(mf + 1) * P],
                                 rhs=msum_sb[:, c:c + 1],
                                 start=(c == 0), stop=(c == FT - 1),
                                 skip_group_check=True)
        hgk_sb = head_pool.tile([P, FT], F32)
        nc.vector.tensor_add(out=hgk_sb, in0=ps_hg, in1=bok_sb)
        nc.sync.dma_start(out=hg_out.rearrange("a (c p) -> p (c a)", p=P),
                          in_=hgk_sb)

        # t1 (k-layout [P, 4]) = wd^T @ hgk + bd
        ps_t = ps_head.tile([P, 4], F32)
        for mt in range(4):
            for c in range(FT):
                nc.tensor.matmul(ps_t[:, mt:mt + 1],
                                 lhsT=wd_sb[:, c, mt * P:(mt + 1) * P],
                                 rhs=hgk_sb[:, c:c + 1],
                                 start=(c == 0), stop=(c == FT - 1),
                                 skip_group_check=True)
        # t1 and t1^2 side by side -> partition-sum via ones-matmul
        t1_sb = head_pool.tile([P, 8], F32)
        nc.vector.tensor_add(out=t1_sb[:, 0:4], in0=ps_t, in1=bdk_sb)
        nc.scalar.activation(out=t1_sb[:, 4:8], in_=t1_sb[:, 0:4],
                             func=AF.Square)

        # LayerNorm over all 512 values (spread across partitions x 4)
        ps_sum = ps_head.tile([1, 8], F32)
        nc.tensor.matmul(ps_sum, lhsT=ones_f32[:, 0:1], rhs=t1_sb,
                         start=True, stop=True)
        sums_sb = head_pool.tile([1, 2], F32)
        nc.vector.tensor_reduce(out=sums_sb[:, 0:1], in_=ps_sum[0:1, 0:4],
                                axis=mybir.AxisListType.X, op=ALU.add)
        nc.vector.tensor_reduce(out=sums_sb[:, 1:2], in_=ps_sum[0:1, 4:8],
                                axis=mybir.AxisListType.X, op=ALU.add)
        ps_bc2 = ps_head.tile([P, 2], F32)
        nc.tensor.matmul(ps_bc2, lhsT=ones_f32[0:1, :], rhs=sums_sb,
                         start=True, stop=True)
        musq_sb = head_pool.tile([P, 2], F32)
        nc.scalar.activation(out=musq_sb, in_=ps_bc2, func=AF.Identity,
                             scale=1.0 / D2)
        mu_sb = musq_sb[:, 0:1]
        msq_sb = musq_sb[:, 1:2]
        var_sb = head_pool.tile([P, 1], F32)
        nc.vector.tensor_mul(out=var_sb, in0=mu_sb, in1=mu_sb)
        nc.vector.tensor_sub(out=var_sb, in0=msq_sb, in1=var_sb)
        nc.scalar.activation(out=var_sb, in_=var_sb, func=AF.Sqrt,
                             bias=eps_sb, scale=1.0)
        rstd_sb = head_pool.tile([P, 1], F32)
        nc.vector.reciprocal(out=rstd_sb, in_=var_sb)
        tn_sb = head_pool.tile([P, 4], F32)
        nc.vector.tensor_scalar(out=tn_sb, in0=t1_sb[:, 0:4], scalar1=mu_sb,
                                scalar2=rstd_sb, op0=ALU.subtract, op1=ALU.mult)
        nc.vector.tensor_mul(out=tn_sb, in0=tn_sb, in1=lngk_sb)
        nc.vector.tensor_add(out=tn_sb, in0=tn_sb, in1=lnbk_sb)
        nc.scalar.activation(out=tn_sb, in_=tn_sb,
                             func=AF.Gelu if cfg.act == "gelu" else AF.Tanh)

        # u = t @ Wh + bh  -> [1, A]
        ps_u = ps_head.tile([1, A], F32)
        for j in range(4):
            nc.tensor.matmul(ps_u, lhsT=tn_sb[:, j:j + 1], rhs=wh_sb[:, j, :],
                             start=(j == 0), stop=(j == 3))
        u_sb = head_pool.tile([1, A], F32)
        nc.vector.tensor_add(out=u_sb, in0=ps_u, in1=bh_sb)

        # expmap0: u * tanh(|u|)/|u|, |u| clamped at 1e-5
        usq_sb = head_pool.tile([1, A], F32)
        ss_sb = head_pool.tile([1, 1], F32)
        nc.scalar.activation(out=usq_sb, in_=u_sb, func=AF.Square,
                             accum_out=ss_sb)
        nn_sb = head_pool.tile([1, 1], F32)
        nc.scalar.activation(out=nn_sb, in_=ss_sb, func=AF.Sqrt,
                             bias=0.0, scale=1.0)
        nc.vector.tensor_scalar_max(out=nn_sb, in0=nn_sb, scalar1=1e-5)
        th_sb = head_pool.tile([1, 1], F32)
        nc.scalar.activation(out=th_sb, in_=nn_sb, func=AF.Tanh)
        ninv_sb = head_pool.tile([1, 1], F32)
        nc.vector.reciprocal(out=ninv_sb, in_=nn_sb)
        fac_sb = head_pool.tile([1, 1], F32)
        nc.vector.tensor_mul(out=fac_sb, in0=th_sb, in1=ninv_sb)
        dout_sb = head_pool.tile([1, A], F32)
        nc.vector.tensor_scalar_mul(out=dout_sb, in0=u_sb, scalar1=fac_sb)
        nc.sync.dma_start(out=def_out, in_=dout_sb)

    ctx.close()


# ----------------------------------------------------------------- entry

_CACHE = {}


def _get_program(cfg):
    key = (cfg.N, cfg.NC)
    if key not in _CACHE:
        _CACHE[key] = build_program(cfg)
    return _CACHE[key]


def kernel(**inputs):
    cfg = Cfg()
    nc = _get_program(cfg)
    in_maps = preprocess(inputs, cfg)
    res = run_bass_kernel_spmd(nc, in_maps, core_ids=list(range(cfg.NC)))
    out = res.results[0]
    return (np.asarray(out["deformation"], np.float32),
            np.asarray(out["h_global"], np.float32))
